# revision 4
# baseline (speedup 1.0000x reference)
"""Trainium2 Bass kernel: 4096x4096 valid cross-correlation with an 11x11
filter + scalar bias, sharded row-wise across 8 NeuronCores.

Strategy
--------
Host-side sharding (halo = overlapping row slices, no collectives): core m
gets input rows [512m, 512m + 522) (core 7 shifted up to stay in bounds)
and produces output rows [512m, 512m + 512).

Per-core compute: conv expressed as banded matmuls on the TensorEngine.
For each kernel column dj, a banded stationary matrix
    B_dj[k, m] = w[k - m, dj]   (0 <= k - m < 11)
contracts over 128 image rows, while column-shifted slices of the image
slab stream as the moving operand:
    out[m, n] += sum_k B_dj[k, m] * x[r0 + k, n0 + n + dj]
Accumulating the 11 dj-shifted matmuls in one PSUM bank yields the full
11x11 correlation for a [118, 512] output tile. float32r runs the PE at
1 cycle/row (vs 4 for plain fp32) with fp32 operands.
"""

import os
import sys

import numpy as np

for _p in ("/opt/trn_rl_repo", "/root/.axon_site/_ro/trn_rl_repo"):
    if os.path.isdir(_p) and _p not in sys.path:
        sys.path.insert(0, _p)

# The device run goes through jax's axon PJRT backend; make sure it is
# visible if jax has not been initialized yet.
_jp = os.environ.get("JAX_PLATFORMS", "")
if "axon" not in _jp.split(","):
    os.environ["JAX_PLATFORMS"] = ("axon," + _jp).strip(",")

import concourse.bacc as bacc
import concourse.bass as bass
import concourse.mybir as mybir
import concourse.tile as tile
from concourse.bass_utils import run_bass_kernel_spmd

H = W = 4096
KH = KW = 11
OH = OW = H - KH + 1  # 4086
NCORES = 8
ROWS_OUT = 512            # output rows per core
ROWS_IN = ROWS_OUT + KH - 1  # 522
M_FULL = 118              # output rows per full slab (contraction K = 128)
# (row offset, M out rows, K contraction rows) per slab; 4*118 + 40 = 512
SLABS = [(0, 118, 128), (118, 118, 128), (236, 118, 128), (354, 118, 128),
         (472, 40, 50)]
BANK_N = [512] * 7 + [OW - 7 * 512]  # 7x512 + 502 = 4086

_cache: dict = {}
LAST_RESULT = None  # BassKernelResults of the most recent device run


def _build():
    f32 = mybir.dt.float32
    f32r = mybir.dt.float32r
    nc = bacc.Bacc("TRN2", target_bir_lowering=False, debug=False,
                   num_devices=NCORES)
    xs_d = nc.dram_tensor("xs", [ROWS_IN, W], f32r, kind="ExternalInput")
    bd_d = nc.dram_tensor("bands", [128, KW * M_FULL], f32r,
                          kind="ExternalInput")
    bias_d = nc.dram_tensor("biasv", [1, 1], f32, kind="ExternalInput")
    out_d = nc.dram_tensor("out", [ROWS_OUT, OW], f32, kind="ExternalOutput")

    with tile.TileContext(nc) as tc:
        with (
            tc.tile_pool(name="bp", bufs=1) as bp,
            tc.tile_pool(name="xp", bufs=3) as xp,
            tc.tile_pool(name="op", bufs=2) as op,
            tc.tile_pool(name="pp", bufs=8, space=bass.MemorySpace.PSUM) as pp,
        ):
            bt = bp.tile([128, KW * M_FULL], f32r, name="bt")
            nc.sync.dma_start(bt[:], bd_d.ap()[:, :])
            bias_bc = bp.tile([128, 1], f32, name="bias_bc")
            nc.sync.dma_start(bias_bc[:], bias_d.ap().to_broadcast((128, 1)))

            for (r0, M, K) in SLABS:
                xt = xp.tile([K, W], f32r, tag="xt", name=f"xt{r0}")
                nc.sync.dma_start(xt[:], xs_d.ap()[r0:r0 + K, :])
                ot = op.tile([M, OW], f32, tag="ot", name=f"ot{r0}")
                for b in range(8):
                    n0 = b * 512
                    N = BANK_N[b]
                    pt = pp.tile([M, 512], f32, tag="ps", name=f"ps{r0}_{b}")
                    for dj in range(KW):
                        nc.tensor.matmul(
                            pt[:, :N],
                            bt[0:K, dj * M_FULL: dj * M_FULL + M],
                            xt[:, n0 + dj: n0 + dj + N],
                            start=(dj == 0),
                            stop=(dj == KW - 1),
                        )
                    nc.scalar.activation(
                        ot[:, n0:n0 + N], pt[:, :N],
                        mybir.ActivationFunctionType.Identity,
                        bias=bias_bc[0:M, :],
                    )
                nc.sync.dma_start(out_d.ap()[r0:r0 + M, :], ot[:])
    nc.compile()
    return nc


def _bands_from_weight(weight: np.ndarray) -> np.ndarray:
    b = np.zeros((128, KW * M_FULL), np.float32)
    for dj in range(KW):
        col = weight[:, dj].astype(np.float32)
        for m in range(M_FULL):
            b[m:m + KH, dj * M_FULL + m] = col
    return b


def kernel(x: np.ndarray, weight: np.ndarray, bias: np.ndarray,
           _trace: bool = False, **_trace_kwargs) -> np.ndarray:
    global LAST_RESULT
    x = np.asarray(x, dtype=np.float32)
    weight = np.asarray(weight, dtype=np.float32)
    bias_v = np.asarray(bias, dtype=np.float32).reshape(1, 1)

    if "nc" not in _cache:
        _cache["nc"] = _build()
    nc = _cache["nc"]

    bands = _bands_from_weight(weight)
    starts = [min(m * ROWS_OUT, H - ROWS_IN) for m in range(NCORES)]
    in_maps = [
        {"xs": np.ascontiguousarray(x[s:s + ROWS_IN]),
         "bands": bands,
         "biasv": bias_v}
        for s in starts
    ]
    res = run_bass_kernel_spmd(nc, in_maps, core_ids=list(range(NCORES)),
                               trace=_trace, **_trace_kwargs)
    LAST_RESULT = res

    out = np.empty((OH, OW), dtype=np.float32)
    for m, s in enumerate(starts):
        r = res.results[m]["out"]
        g0 = m * ROWS_OUT           # first global output row wanted from core m
        keep0 = g0 - s              # 0 for cores 0-6, 10 for core 7
        take = min(ROWS_OUT - keep0, OH - g0)
        out[g0:g0 + take] = r[keep0:keep0 + take]
    return out


# revision 6
# speedup vs baseline: 1.0076x; 1.0076x over previous
"""Trainium2 Bass kernel: 4096x4096 valid cross-correlation with an 11x11
filter + scalar bias, sharded row-wise across 8 NeuronCores.

Strategy
--------
Host-side sharding (halo = overlapping row slices, no collectives): core m
gets input rows [512m, 512m + 522) (core 7 shifted up to stay in bounds)
and produces output rows [512m, 512m + 512).

Per-core compute: conv expressed as banded matmuls on the TensorEngine.
For each kernel column dj, a banded stationary matrix
    B_dj[k, m] = w[k - m, dj]   (0 <= k - m < 11)
contracts over 128 image rows, while column-shifted slices of the image
slab stream as the moving operand:
    out[m, n] += sum_k B_dj[k, m] * x[r0 + k, n0 + n + dj]
Accumulating the 11 dj-shifted matmuls in one PSUM bank yields the full
11x11 correlation for a [118, 512] output tile. float32r runs the PE at
1 cycle/row (vs 4 for plain fp32) with fp32 operands.
"""

import os
import sys

import numpy as np

for _p in ("/opt/trn_rl_repo", "/root/.axon_site/_ro/trn_rl_repo"):
    if os.path.isdir(_p) and _p not in sys.path:
        sys.path.insert(0, _p)

# The device run goes through jax's axon PJRT backend; make sure it is
# visible if jax has not been initialized yet.
_jp = os.environ.get("JAX_PLATFORMS", "")
if "axon" not in _jp.split(","):
    os.environ["JAX_PLATFORMS"] = ("axon," + _jp).strip(",")

import concourse.bacc as bacc
import concourse.bass as bass
import concourse.mybir as mybir
import concourse.tile as tile
from concourse.bass_utils import run_bass_kernel_spmd

H = W = 4096
KH = KW = 11
OH = OW = H - KH + 1  # 4086
NCORES = 8
ROWS_OUT = 512            # output rows per core
ROWS_IN = ROWS_OUT + KH - 1  # 522
M_FULL = 118              # output rows per full slab (contraction K = 128)
# (row offset, M out rows, K contraction rows) per slab; 4*118 + 40 = 512
SLABS = [(0, 118, 128), (118, 118, 128), (236, 118, 128), (354, 118, 128),
         (472, 40, 50)]
BANK_N = [512] * 7 + [OW - 7 * 512]  # 7x512 + 502 = 4086

_cache: dict = {}
LAST_RESULT = None  # BassKernelResults of the most recent device run


def _build():
    f32 = mybir.dt.float32
    f32r = mybir.dt.float32r
    nc = bacc.Bacc("TRN2", target_bir_lowering=False, debug=False,
                   num_devices=NCORES)
    xs_d = nc.dram_tensor("xs", [ROWS_IN, W], f32r, kind="ExternalInput")
    bd_d = nc.dram_tensor("bands", [128, KW * M_FULL], f32r,
                          kind="ExternalInput")
    bias_d = nc.dram_tensor("biasv", [1, 1], f32, kind="ExternalInput")
    out_d = nc.dram_tensor("out", [ROWS_OUT, OW], f32, kind="ExternalOutput")

    with tile.TileContext(nc) as tc:
        with (
            tc.tile_pool(name="bp", bufs=1) as bp,
            tc.tile_pool(name="xp", bufs=3) as xp,
            tc.tile_pool(name="op", bufs=2) as op,
            tc.tile_pool(name="pp", bufs=8, space=bass.MemorySpace.PSUM) as pp,
        ):
            def dma_rows(dst, src, rows, nsplit):
                # split a [rows, ...] transfer into row chunks so the HWDGE
                # fans it across more DMA engines (one 16KB packet per row;
                # a single dma_start only engages ~2 engines)
                step = (rows + nsplit - 1) // nsplit
                for c0 in range(0, rows, step):
                    c1 = min(c0 + step, rows)
                    nc.sync.dma_start(dst[c0:c1], src[c0:c1])

            bt = bp.tile([128, KW * M_FULL], f32r, name="bt")
            nc.sync.dma_start(bt[:], bd_d.ap()[:, :])
            bias_bc = bp.tile([128, 1], f32, name="bias_bc")
            nc.sync.dma_start(bias_bc[:], bias_d.ap().to_broadcast((128, 1)))

            for (r0, M, K) in SLABS:
                xt = xp.tile([K, W], f32r, tag="xt", name=f"xt{r0}")
                dma_rows(xt, xs_d.ap()[r0:r0 + K, :], K, 4)
                ot = op.tile([M, OW], f32, tag="ot", name=f"ot{r0}")
                for b in range(8):
                    n0 = b * 512
                    N = BANK_N[b]
                    pt = pp.tile([M, 512], f32, tag="ps", name=f"ps{r0}_{b}")
                    for dj in range(KW):
                        nc.tensor.matmul(
                            pt[:, :N],
                            bt[0:K, dj * M_FULL: dj * M_FULL + M],
                            xt[:, n0 + dj: n0 + dj + N],
                            start=(dj == 0),
                            stop=(dj == KW - 1),
                        )
                    nc.scalar.activation(
                        ot[:, n0:n0 + N], pt[:, :N],
                        mybir.ActivationFunctionType.Identity,
                        bias=bias_bc[0:M, :],
                    )
                dma_rows(out_d.ap()[r0:r0 + M, :], ot[:], M, 4)
    nc.compile()
    return nc


def _bands_from_weight(weight: np.ndarray) -> np.ndarray:
    b = np.zeros((128, KW * M_FULL), np.float32)
    for dj in range(KW):
        col = weight[:, dj].astype(np.float32)
        for m in range(M_FULL):
            b[m:m + KH, dj * M_FULL + m] = col
    return b


def kernel(x: np.ndarray, weight: np.ndarray, bias: np.ndarray,
           _trace: bool = False, **_trace_kwargs) -> np.ndarray:
    global LAST_RESULT
    x = np.asarray(x, dtype=np.float32)
    weight = np.asarray(weight, dtype=np.float32)
    bias_v = np.asarray(bias, dtype=np.float32).reshape(1, 1)

    if "nc" not in _cache:
        _cache["nc"] = _build()
    nc = _cache["nc"]

    bands = _bands_from_weight(weight)
    starts = [min(m * ROWS_OUT, H - ROWS_IN) for m in range(NCORES)]
    in_maps = [
        {"xs": np.ascontiguousarray(x[s:s + ROWS_IN]),
         "bands": bands,
         "biasv": bias_v}
        for s in starts
    ]
    res = run_bass_kernel_spmd(nc, in_maps, core_ids=list(range(NCORES)),
                               trace=_trace, **_trace_kwargs)
    LAST_RESULT = res

    out = np.empty((OH, OW), dtype=np.float32)
    for m, s in enumerate(starts):
        r = res.results[m]["out"]
        g0 = m * ROWS_OUT           # first global output row wanted from core m
        keep0 = g0 - s              # 0 for cores 0-6, 10 for core 7
        take = min(ROWS_OUT - keep0, OH - g0)
        out[g0:g0 + take] = r[keep0:keep0 + take]
    return out


# revision 8
# speedup vs baseline: 1.2979x; 1.2880x over previous
"""Trainium2 Bass kernel: 4096x4096 valid cross-correlation with an 11x11
filter + scalar bias, sharded row-wise across 8 NeuronCores.

Strategy
--------
Host-side sharding (halo = overlapping row slices, no collectives): core m
gets input rows [512m, 512m + 522) (core 7 shifted up to stay in bounds)
and produces output rows [512m, 512m + 512).

Per-core compute: conv expressed as banded matmuls on the TensorEngine.
For each kernel column dj, a banded stationary matrix
    B_dj[k, m] = w[k - m, dj]   (0 <= k - m < 11)
contracts over 128 image rows, while column-shifted slices of the image
slab stream as the moving operand:
    out[m, n] += sum_k B_dj[k, m] * x[r0 + k, n0 + n + dj]
Accumulating the 11 dj-shifted matmuls in one PSUM bank yields the full
11x11 correlation for a [118, 512] output tile. float32r runs the PE at
1 cycle/row (vs 4 for plain fp32) with fp32 operands.
"""

import os
import sys

import numpy as np

for _p in ("/opt/trn_rl_repo", "/root/.axon_site/_ro/trn_rl_repo"):
    if os.path.isdir(_p) and _p not in sys.path:
        sys.path.insert(0, _p)

# The device run goes through jax's axon PJRT backend; make sure it is
# visible if jax has not been initialized yet.
_jp = os.environ.get("JAX_PLATFORMS", "")
if "axon" not in _jp.split(","):
    os.environ["JAX_PLATFORMS"] = ("axon," + _jp).strip(",")

import concourse.bacc as bacc
import concourse.bass as bass
import concourse.mybir as mybir
import concourse.tile as tile
from concourse.bass_utils import run_bass_kernel_spmd

H = W = 4096
KH = KW = 11
OH = OW = H - KH + 1  # 4086
NCORES = 8
ROWS_OUT = 512            # output rows per core
ROWS_IN = ROWS_OUT + KH - 1  # 522
M_FULL = 118              # output rows per full slab (contraction K = 128)
# (row offset, M out rows, K contraction rows) per slab; 4*118 + 40 = 512
SLABS = [(0, 118, 128), (118, 118, 128), (236, 118, 128), (354, 118, 128),
         (472, 40, 50)]
BANK_N = [512] * 7 + [OW - 7 * 512]  # 7x512 + 502 = 4086

_cache: dict = {}
LAST_RESULT = None  # BassKernelResults of the most recent device run


def _build():
    f32 = mybir.dt.float32
    f32r = mybir.dt.float32r
    nc = bacc.Bacc("TRN2", target_bir_lowering=False, debug=False,
                   num_devices=NCORES)
    xs_d = nc.dram_tensor("xs", [ROWS_IN, W], f32r, kind="ExternalInput")
    bd_d = nc.dram_tensor("bands", [128, KW * M_FULL], f32r,
                          kind="ExternalInput")
    bias_d = nc.dram_tensor("biasv", [1, 1], f32, kind="ExternalInput")
    out_d = nc.dram_tensor("out", [ROWS_OUT, OW], f32, kind="ExternalOutput")

    with tile.TileContext(nc) as tc:
        with (
            tc.tile_pool(name="bp", bufs=1) as bp,
            tc.tile_pool(name="xp", bufs=1) as xp,
            tc.tile_pool(name="op", bufs=3) as op,
            tc.tile_pool(name="pp", bufs=8, space=bass.MemorySpace.PSUM) as pp,
        ):
            def dma_rows(dst, src, rows, nsplit):
                # split a [rows, ...] transfer into row chunks so the HWDGE
                # fans it across more DMA engines (one 16KB packet per row;
                # a single dma_start only engages ~2 engines)
                step = (rows + nsplit - 1) // nsplit
                for c0 in range(0, rows, step):
                    c1 = min(c0 + step, rows)
                    nc.sync.dma_start(dst[c0:c1], src[c0:c1])

            bt = bp.tile([128, KW * M_FULL], f32r, name="bt")
            nc.sync.dma_start(bt[:], bd_d.ap()[:, :])
            bias_bc = bp.tile([128, 1], f32, name="bias_bc")
            nc.sync.dma_start(bias_bc[:], bias_d.ap().to_broadcast((128, 1)))

            # all slab loads issued upfront: every xt is SBUF-resident (5 x
            # 2MB), so no load trigger ever queues behind a store's wait on
            # the Sync engine. Slab 0 is split widest to start the PE soonest.
            xts = {}
            for si, (r0, M, K) in enumerate(SLABS):
                xt = xp.tile([K, W], f32r, tag=f"xt{si}", name=f"xt{si}")
                dma_rows(xt, xs_d.ap()[r0:r0 + K, :], K, 8 if si == 0 else 4)
                xts[si] = xt

            for si, (r0, M, K) in enumerate(SLABS):
                xt = xts[si]
                ot = op.tile([M, OW], f32, tag="ot", name=f"ot{r0}")
                for b in range(8):
                    n0 = b * 512
                    N = BANK_N[b]
                    pt = pp.tile([M, 512], f32, tag="ps", name=f"ps{r0}_{b}")
                    for dj in range(KW):
                        nc.tensor.matmul(
                            pt[:, :N],
                            bt[0:K, dj * M_FULL: dj * M_FULL + M],
                            xt[:, n0 + dj: n0 + dj + N],
                            start=(dj == 0),
                            stop=(dj == KW - 1),
                        )
                    nc.scalar.activation(
                        ot[:, n0:n0 + N], pt[:, :N],
                        mybir.ActivationFunctionType.Identity,
                        bias=bias_bc[0:M, :],
                    )
                dma_rows(out_d.ap()[r0:r0 + M, :], ot[:], M, 4)
    nc.compile()
    return nc


def _bands_from_weight(weight: np.ndarray) -> np.ndarray:
    b = np.zeros((128, KW * M_FULL), np.float32)
    for dj in range(KW):
        col = weight[:, dj].astype(np.float32)
        for m in range(M_FULL):
            b[m:m + KH, dj * M_FULL + m] = col
    return b


def kernel(x: np.ndarray, weight: np.ndarray, bias: np.ndarray,
           _trace: bool = False, **_trace_kwargs) -> np.ndarray:
    global LAST_RESULT
    x = np.asarray(x, dtype=np.float32)
    weight = np.asarray(weight, dtype=np.float32)
    bias_v = np.asarray(bias, dtype=np.float32).reshape(1, 1)

    if "nc" not in _cache:
        _cache["nc"] = _build()
    nc = _cache["nc"]

    bands = _bands_from_weight(weight)
    starts = [min(m * ROWS_OUT, H - ROWS_IN) for m in range(NCORES)]
    in_maps = [
        {"xs": np.ascontiguousarray(x[s:s + ROWS_IN]),
         "bands": bands,
         "biasv": bias_v}
        for s in starts
    ]
    res = run_bass_kernel_spmd(nc, in_maps, core_ids=list(range(NCORES)),
                               trace=_trace, **_trace_kwargs)
    LAST_RESULT = res

    out = np.empty((OH, OW), dtype=np.float32)
    for m, s in enumerate(starts):
        r = res.results[m]["out"]
        g0 = m * ROWS_OUT           # first global output row wanted from core m
        keep0 = g0 - s              # 0 for cores 0-6, 10 for core 7
        take = min(ROWS_OUT - keep0, OH - g0)
        out[g0:g0 + take] = r[keep0:keep0 + take]
    return out
